# revision 10
# baseline (speedup 1.0000x reference)
"""Self-contained Trainium2 Bass kernel for nn_Coconut (8 NeuronCores).

Math (validated vs reference in fp32 numpy to ~1e-6 rel):
  The reference's multi-pass latent recompute == ONE causal forward with a
  KV cache:
    chunk A: tokens 0..255 (2 layers + lnf)        -> H[:, 0:256]
    7 serial decode steps t=256..262:  emb[t+1] = lnf(x_t) + wpe[t+1]
    chunk B: tokens 256..511 (reuses KV of 0..255) -> H[:, 256:512]
    logits[t] = H[t] @ wte.T ; shifted CE loss (ignore -100).

Sharding: 8 cores = 4 batch rows x 2 vocab halves. Each core of a pair
duplicates its row's transformer (cheap); the vocab GEMM is column-parallel
(16000 vocab per core) and fused with sum-exp for the loss. Host combines
halves and computes the scalar loss + assembles full logits.

Layouts (per core): feature-major activations xT[p, ci, t] with d=ci*128+p;
kcache same; vcache token-major [p_tok, tb, 65*h + dh] with a ones column
per head (softmax denominator falls out of the same matmul as o^T).
Weights fp16 (matmul inputs), accumulation/residual f32.
"""
import numpy as np

B, S, D, NH, NL, V = 4, 512, 512, 8, 2, 32000
DH = D // NH
L0, NLAT = 256, 8
VH = V // 2
NVB, NV = 32, 500
P = 128
NCORES = 8

_CACHE = {}


# ---------------------------------------------------------------- host packing

def _featmajor(a, dt):
    """[Din, N...] -> [128, Din//128, N...] with d = ci*128+p."""
    d0 = a.shape[0]
    return np.ascontiguousarray(
        a.reshape(d0 // P, P, *a.shape[1:]).transpose(1, 0, *range(2, a.ndim + 1))
    ).astype(dt)


def pack_inputs(input_ids, attention_mask, labels, position_ids, params):
    f32, f16 = np.float32, np.float16
    wte = np.asarray(params["wte"], f32)
    wpe = np.asarray(params["wpe"], f32)
    ids = np.asarray(input_ids)
    pos = np.asarray(position_ids)
    lat = np.nonzero(ids[0] == 31998)[0]
    assert len(lat) == NLAT and lat[0] == L0 and lat[-1] == L0 + NLAT - 1, lat
    emb = wte[ids] + wpe[pos]                      # [B,S,D] f32

    com = {}
    for li, bl in enumerate(params["blocks"]):
        wqkv = np.asarray(bl["w_qkv"], f32)
        com[f"qkv{li}"] = _featmajor(wqkv, f16)
        com[f"bqkv{li}"] = np.asarray(bl["b_qkv"], f32)[:1024].reshape(8, P).T.astype(f32)
        com[f"bv{li}"] = np.ascontiguousarray(
            np.asarray(bl["b_qkv"], f32)[1024:].reshape(1, 512))
        com[f"wo{li}"] = _featmajor(np.asarray(bl["w_o"], f32), f16)
        com[f"bo{li}"] = np.asarray(bl["b_o"], f32).reshape(4, P).T.astype(f32)
        com[f"wfc{li}"] = _featmajor(np.asarray(bl["w_fc"], f32), f16)
        com[f"bfc{li}"] = np.asarray(bl["b_fc"], f32).reshape(16, P).T.astype(f32)
        com[f"wpr{li}"] = _featmajor(np.asarray(bl["w_pr"], f32), f16)
        com[f"bpr{li}"] = np.asarray(bl["b_pr"], f32).reshape(4, P).T.astype(f32)
        for nm in ("ln1_g", "ln1_b", "ln2_g", "ln2_b"):
            com[f"{nm.replace('_', '')}{li}"] = (
                np.asarray(bl[nm], f32).reshape(4, P).T.astype(f32))
    com["lnfg"] = np.asarray(params["lnf_g"], f32).reshape(4, P).T.astype(f32)
    com["lnfb"] = np.asarray(params["lnf_b"], f32).reshape(4, P).T.astype(f32)

    wteT = {}
    for half in range(2):
        wh = wte[half * VH:(half + 1) * VH]
        wteT[half] = np.ascontiguousarray(
            wh.reshape(NVB, NV, 4, P).transpose(3, 0, 2, 1)).astype(f16)

    in_maps = []
    for c in range(NCORES):
        r, half = c // 2, c % 2
        m = dict(com)
        m["emb"] = _featmajor(np.ascontiguousarray(emb[r].T), f32)
        m["wpelat"] = _featmajor(
            np.ascontiguousarray(wpe[pos[r][L0:L0 + NLAT]].T), f32)
        m["wte"] = wteT[half]
        in_maps.append(m)
    return in_maps


# ---------------------------------------------------------------- kernel build

def build_nc(debug_outputs=False, sim_gelu=False):
    from contextlib import ExitStack
    import concourse.bacc as bacc
    import concourse.mybir as mybir
    import concourse.tile as tile

    F32, F16 = mybir.dt.float32, mybir.dt.float16
    AF = mybir.ActivationFunctionType
    ALU = mybir.AluOpType

    nc = bacc.Bacc("TRN2", target_bir_lowering=False, debug=False,
                   num_devices=NCORES)
    names = {}

    with tile.TileContext(nc) as tc, ExitStack() as ctx:
        dram = ctx.enter_context(tc.tile_pool(name="dram", bufs=1, space="DRAM"))
        cw = ctx.enter_context(tc.tile_pool(name="cw", bufs=1))
        st = ctx.enter_context(tc.tile_pool(name="st", bufs=1))
        wk = ctx.enter_context(tc.tile_pool(name="wk", bufs=2))
        vp = ctx.enter_context(tc.tile_pool(name="vp", bufs=3))
        ps = ctx.enter_context(tc.tile_pool(name="ps", bufs=2, space="PSUM"))

        def din(name, shape, dt):
            h = dram.tile(list(shape), dt, kind="ExternalInput", name=name)
            names[name] = h.name
            return h

        def dout(name, shape, dt):
            h = dram.tile(list(shape), dt, kind="ExternalOutput", name=name)
            names[name] = h.name
            return h

        d_emb = din("emb", (P, 4, S), F32)
        d_wpelat = din("wpelat", (P, 4, NLAT), F32)
        d_wte = din("wte", (P, NVB, 4, NV), F16)
        dW = {}
        for li in range(NL):
            dW[f"qkv{li}"] = din(f"qkv{li}", (P, 4, 1536), F16)
            dW[f"bqkv{li}"] = din(f"bqkv{li}", (P, 8), F32)
            dW[f"bv{li}"] = din(f"bv{li}", (1, 512), F32)
            dW[f"wo{li}"] = din(f"wo{li}", (P, 4, 512), F16)
            dW[f"bo{li}"] = din(f"bo{li}", (P, 4), F32)
            dW[f"wfc{li}"] = din(f"wfc{li}", (P, 4, 2048), F16)
            dW[f"bfc{li}"] = din(f"bfc{li}", (P, 16), F32)
            dW[f"wpr{li}"] = din(f"wpr{li}", (P, 16, 512), F16)
            dW[f"bpr{li}"] = din(f"bpr{li}", (P, 4), F32)
            for nm in ("ln1g", "ln1b", "ln2g", "ln2b"):
                dW[f"{nm}{li}"] = din(f"{nm}{li}", (P, 4), F32)
        d_lnfg = din("lnfg", (P, 4), F32)
        d_lnfb = din("lnfb", (P, 4), F32)
        d_logits = dout("logits", (P, 4, VH), F32)
        d_sumexp = dout("sumexp", (P, 4), F32)
        if debug_outputs:
            d_ht = dout("dbg_ht", (P, 4, S), F16)
            d_x = dout("dbg_x", (P, 4, S), F32)

        # ---- SBUF consts
        W = {}
        for k, d in dW.items():
            W[k] = cw.tile(list(d.shape), d.dtype, name=f"sb_{k}")
            nc.sync.dma_start(W[k][:], d[:])
        lnfg = cw.tile([P, 4], F32, name="sb_lnfg")
        lnfb = cw.tile([P, 4], F32, name="sb_lnfb")
        wpelat = cw.tile([P, 4, NLAT], F32, name="sb_wpelat")
        nc.sync.dma_start(lnfg[:], d_lnfg[:])
        nc.sync.dma_start(lnfb[:], d_lnfb[:])
        nc.sync.dma_start(wpelat[:], d_wpelat[:])
        ones1 = cw.tile([P, 1], F32, name="ones1")
        onesr = cw.tile([1, P], F32, name="onesr")
        nc.gpsimd.memset(ones1[:], 1.0)
        nc.gpsimd.memset(onesr[:], 1.0)
        I32 = mybir.dt.int32
        qc = cw.tile([P, 512], I32, name="qc")
        nc.gpsimd.memset(qc[:], 0x5F3759DF)

        # ---- state
        x = st.tile([P, 4, S], F32, name="x")
        nc.sync.dma_start(x[:], d_emb[:])
        kc = [st.tile([P, 4, S], F16, name=f"kc{li}") for li in range(NL)]
        vc = [st.tile([P, 4, 8 * 65], F16, name=f"vc{li}") for li in range(NL)]
        for li in range(NL):
            nc.gpsimd.memset(vc[li][:], 1.0)
        ht = st.tile([P, 4, S], F16, name="ht")
        seacc = st.tile([P, 4, NVB], F32, name="seacc")
        vstage = [st.tile([1, 8 * 65], F16, name=f"vstage{li}")
                  for li in range(NL)]
        for li in range(NL):
            nc.gpsimd.memset(vstage[li][:], 1.0)

        def vrows(li, tb):
            """vcache token-block tb viewed as [128, 8, 65]."""
            return vc[li][:, tb, :].rearrange("p (h e) -> p h e", h=8)

        # ================= layernorm =================
        def layernorm(src_fn, nq, out_cb, pref):
            """src_fn(ci)->AP [128,nq] f32. out_cb(ci, t1) consumes normalized
            (pre-gain) f32 [128,nq]."""
            sqs = []
            for ci in range(4):
                sq = wk.tile([P, nq], F32, name=f"{pref}sq{ci}", tag="ln_sq",
                             bufs=4)
                nc.vector.tensor_tensor(sq[:], src_fn(ci), src_fn(ci), ALU.mult)
                sqs.append(sq)
            ps_sum = ps.tile([1, nq], F32, name=f"{pref}pssum", tag="small")
            ps_ss = ps.tile([1, nq], F32, name=f"{pref}psss", tag="small")
            for ci in range(4):
                nc.tensor.matmul(ps_sum[:], ones1[:], src_fn(ci),
                                 start=(ci == 0), stop=(ci == 3))
            for ci in range(4):
                nc.tensor.matmul(ps_ss[:], ones1[:], sqs[ci][:],
                                 start=(ci == 0), stop=(ci == 3))
            m = wk.tile([1, nq], F32, name=f"{pref}m", tag="ln_m")
            ssv = wk.tile([1, nq], F32, name=f"{pref}ss", tag="ln_ss")
            nc.vector.tensor_scalar_mul(m[:], ps_sum[:], 1.0 / D)
            nc.vector.tensor_scalar_mul(ssv[:], ps_ss[:], 1.0 / D)
            var = wk.tile([1, nq], F32, name=f"{pref}var", tag="ln_var")
            nc.vector.tensor_tensor(var[:], m[:], m[:], ALU.mult)
            nc.vector.tensor_tensor(var[:], ssv[:], var[:], ALU.subtract)
            nc.vector.tensor_scalar_add(var[:], var[:], 1e-5)
            ps_mb = ps.tile([P, nq], F32, name=f"{pref}psmb", tag="small")
            ps_rb = ps.tile([P, nq], F32, name=f"{pref}psrb", tag="small")
            nc.tensor.matmul(ps_mb[:], onesr[:], m[:], start=True, stop=True)
            nc.tensor.matmul(ps_rb[:], onesr[:], var[:], start=True, stop=True)
            mb = wk.tile([P, nq], F32, name=f"{pref}mb", tag="ln_mb", bufs=1)
            vb = wk.tile([P, nq], F32, name=f"{pref}vb", tag="ln_vb", bufs=1)
            nc.vector.tensor_copy(mb[:], ps_mb[:])
            nc.vector.tensor_copy(vb[:], ps_rb[:])
            # rb = 1/sqrt(vb): quake seed + 2 Newton iterations (DVE only,
            # avoids an ACT table switch to sqrt_and_others)
            rb = wk.tile([P, nq], F32, name=f"{pref}rb", tag="ln_rb", bufs=1)
            shi = wk.tile([P, nq], mybir.dt.int32, name=f"{pref}shi",
                          tag="ln_shi")
            tmp = wk.tile([P, nq], F32, name=f"{pref}tmp", tag="ln_tmp")
            nc.vector.tensor_scalar(shi[:], vb[:].bitcast(mybir.dt.int32), 1,
                                    None, ALU.arith_shift_right)
            nc.vector.tensor_tensor(shi[:], qc[:, 0:nq], shi[:], ALU.subtract)
            y0 = shi[:].bitcast(F32)
            nc.vector.tensor_tensor(tmp[:], y0, y0, ALU.mult)
            nc.vector.tensor_tensor(tmp[:], tmp[:], vb[:], ALU.mult)
            nc.vector.tensor_scalar(tmp[:], tmp[:], -0.5, 1.5, ALU.mult, ALU.add)
            nc.vector.tensor_tensor(rb[:], y0, tmp[:], ALU.mult)
            nc.vector.tensor_tensor(tmp[:], rb[:], rb[:], ALU.mult)
            nc.vector.tensor_tensor(tmp[:], tmp[:], vb[:], ALU.mult)
            nc.vector.tensor_scalar(tmp[:], tmp[:], -0.5, 1.5, ALU.mult, ALU.add)
            nc.vector.tensor_tensor(rb[:], rb[:], tmp[:], ALU.mult)
            for ci in range(4):
                t1 = wk.tile([P, nq], F32, name=f"{pref}t1{ci}", tag="ln_t1")
                nc.vector.tensor_tensor(t1[:], src_fn(ci), mb[:], ALU.subtract)
                nc.vector.tensor_tensor(t1[:], t1[:], rb[:], ALU.mult)
                out_cb(ci, t1)

        def ln_to16(src_fn, nq, g, b, pref, tag="h16"):
            out16 = wk.tile([P, 4, nq], F16, name=f"{pref}o", tag=tag)

            def cb(ci, t1):
                nc.scalar.activation(out16[:, ci, :], t1[:], AF.Identity,
                                     scale=g[:, ci:ci + 1], bias=b[:, ci:ci + 1])
            layernorm(src_fn, nq, cb, pref)
            return out16

        # gelu(z) = z / (1 + exp(-2c*(z + a z^3))), c=sqrt(2/pi), a=0.044715
        # (exact tanh-approx gelu, using only exp_and_others ACT table)
        def compose_gelu(za, oa, nfree, pref):
            z2 = wk.tile([P, nfree], F32, name=f"{pref}gz2", tag="gel_t",
                         padded_shape=[P, 256])
            nc.vector.tensor_tensor(z2[:, 0:nfree], za, za, ALU.mult)
            nc.vector.tensor_scalar(z2[:, 0:nfree], z2[:, 0:nfree], 0.044715,
                                    1.0, ALU.mult, ALU.add)
            nc.vector.tensor_tensor(z2[:, 0:nfree], za, z2[:, 0:nfree],
                                    ALU.mult)
            ge = wk.tile([P, nfree], F32, name=f"{pref}ge", tag="gel_e",
                         padded_shape=[P, 256])
            nc.scalar.activation(ge[:, 0:nfree], z2[:, 0:nfree], AF.Exp,
                                 scale=-1.5957691216057308)
            nc.vector.tensor_scalar_add(ge[:, 0:nfree], ge[:, 0:nfree], 1.0)
            nc.vector.reciprocal(ge[:, 0:nfree], ge[:, 0:nfree])
            nc.vector.tensor_tensor(oa, za, ge[:, 0:nfree], ALU.mult)

        # ================= transformer layer on a chunk =================
        def chunk_layer(li, q0, nq, pref):
            nqb = nq // P
            nkb = (q0 + nq) // P
            h16 = ln_to16(lambda ci: x[:, ci, q0:q0 + nq], nq,
                          W[f"ln1g{li}"], W[f"ln1b{li}"], f"{pref}n1")
            qkv = W[f"qkv{li}"]
            qt16 = wk.tile([P, 4, nq], F16, name=f"{pref}q", tag="qt16")
            for mb in range(8):
                pmm = ps.tile([P, nq], F32, name=f"{pref}qkv{mb}", tag="mm")
                for ci in range(4):
                    nc.tensor.matmul(pmm[:], qkv[:, ci, mb * P:(mb + 1) * P],
                                     h16[:, ci, :], start=(ci == 0),
                                     stop=(ci == 3))
                dst = (qt16[:, mb, :] if mb < 4
                       else kc[li][:, mb - 4, q0:q0 + nq])
                nc.scalar.activation(dst, pmm[:], AF.Identity,
                                     bias=W[f"bqkv{li}"][:, mb:mb + 1])
            # v (token-major) with broadcast bias
            ps_bv = ps.tile([P, 512], F32, name=f"{pref}psbv", tag="small")
            nc.tensor.matmul(ps_bv[:], onesr[:], W[f"bv{li}"][:],
                             start=True, stop=True)
            bvb = wk.tile([P, 512], F32, name=f"{pref}bvb", tag="bvb", bufs=1)
            nc.vector.tensor_copy(bvb[:], ps_bv[:])
            for tb in range(nqb):
                pv = ps.tile([P, 512], F32, name=f"{pref}v{tb}", tag="mm")
                for ci in range(4):
                    nc.tensor.matmul(pv[:], h16[:, ci, tb * P:(tb + 1) * P],
                                     qkv[:, ci, 1024:1536],
                                     start=(ci == 0), stop=(ci == 3))
                nc.vector.tensor_tensor(vrows(li, q0 // P + tb)[:, :, 0:64],
                                        pv[:], bvb[:], ALU.add)
            # attention
            ot16 = wk.tile([P, 4, nq], F16, name=f"{pref}ot", tag="ot16")
            for h in range(NH):
                cih, p0 = h // 2, (h % 2) * 64
                pso = ps.tile([65, nq], F32, name=f"{pref}o{h}", tag="attn_o")
                for kb in range(nkb):
                    qs = max(0, kb * P - q0)
                    Nq = nq - qs
                    pss = ps.tile([P, Nq], F32, name=f"{pref}s{h}_{kb}",
                                  tag="attn_s")
                    nc.tensor.matmul(
                        pss[:], kc[li][p0:p0 + 64, cih, kb * P:(kb + 1) * P],
                        qt16[p0:p0 + 64, cih, qs:nq], start=True, stop=True)
                    et = wk.tile([P, Nq], F16, name=f"{pref}e{h}_{kb}",
                                 tag="et", bufs=2)
                    nc.scalar.activation(et[:], pss[:], AF.Exp, scale=0.125)
                    if kb * P >= q0 + qs:  # diagonal block: causal zeroing
                        nc.gpsimd.affine_select(et[:], et[:],
                                                pattern=[[1, Nq]],
                                                compare_op=ALU.is_ge,
                                                fill=0.0, base=0,
                                                channel_multiplier=-1)
                    nc.tensor.matmul(pso[:, qs:nq],
                                     vc[li][:, kb, 65 * h:65 * h + 65],
                                     et[:], start=(kb == 0), stop=(kb == nkb - 1))
                rec = wk.tile([1, nq], F32, name=f"{pref}rc{h}", tag="rec")
                nc.vector.reciprocal(rec[:], pso[64:65, :])
                ps_rb2 = ps.tile([64, nq], F32, name=f"{pref}rb{h}", tag="small")
                nc.tensor.matmul(ps_rb2[:], onesr[0:1, 0:64], rec[:],
                                 start=True, stop=True)
                rbc = wk.tile([64, nq], F32, name=f"{pref}rbc{h}", tag="rbc")
                nc.vector.tensor_copy(rbc[:], ps_rb2[:])
                nc.vector.tensor_tensor(ot16[p0:p0 + 64, cih, :],
                                        pso[0:64, :], rbc[:], ALU.mult)
            # o proj + residual
            for ci in range(4):
                pmm = ps.tile([P, nq], F32, name=f"{pref}op{ci}", tag="mm")
                for cik in range(4):
                    nc.tensor.matmul(pmm[:],
                                     W[f"wo{li}"][:, cik, ci * P:(ci + 1) * P],
                                     ot16[:, cik, :], start=(cik == 0),
                                     stop=(cik == 3))
                res = wk.tile([P, nq], F32, name=f"{pref}or{ci}", tag="resid")
                nc.scalar.activation(res[:], pmm[:], AF.Identity,
                                     bias=W[f"bo{li}"][:, ci:ci + 1])
                nc.vector.tensor_tensor(x[:, ci, q0:q0 + nq],
                                        x[:, ci, q0:q0 + nq], res[:], ALU.add)
            # mlp
            h2 = ln_to16(lambda ci: x[:, ci, q0:q0 + nq], nq,
                         W[f"ln2g{li}"], W[f"ln2b{li}"], f"{pref}n2")
            fc16 = wk.tile([P, 16, nq], F16, name=f"{pref}fc", tag="fc16",
                           bufs=1)
            for mb in range(16):
                pmm = ps.tile([P, nq], F32, name=f"{pref}fc{mb}", tag="mm")
                for ci in range(4):
                    nc.tensor.matmul(pmm[:],
                                     W[f"wfc{li}"][:, ci, mb * P:(mb + 1) * P],
                                     h2[:, ci, :], start=(ci == 0),
                                     stop=(ci == 3))
                if sim_gelu:
                    zc = wk.tile([P, nq], F32, name=f"{pref}zc{mb}", tag="zc")
                    nc.scalar.activation(zc[:], pmm[:], AF.Identity,
                                         bias=W[f"bfc{li}"][:, mb:mb + 1])
                    compose_gelu(zc[:], fc16[:, mb, :], nq, f"{pref}g{mb}")
                else:
                    nc.scalar.activation(fc16[:, mb, :], pmm[:],
                                         AF.Gelu_apprx_tanh,
                                         bias=W[f"bfc{li}"][:, mb:mb + 1])
            for ci in range(4):
                pmm = ps.tile([P, nq], F32, name=f"{pref}pr{ci}", tag="mm")
                for mb in range(16):
                    nc.tensor.matmul(pmm[:],
                                     W[f"wpr{li}"][:, mb, ci * P:(ci + 1) * P],
                                     fc16[:, mb, :], start=(mb == 0),
                                     stop=(mb == 15))
                res = wk.tile([P, nq], F32, name=f"{pref}prr{ci}", tag="resid")
                nc.scalar.activation(res[:], pmm[:], AF.Identity,
                                     bias=W[f"bpr{li}"][:, ci:ci + 1])
                nc.vector.tensor_tensor(x[:, ci, q0:q0 + nq],
                                        x[:, ci, q0:q0 + nq], res[:], ALU.add)

        def chunk_lnf(q0, nq, pref, scatter_col=None):
            """lnf -> ht cols q0..q0+nq (fp16). If scatter_col is not None,
            also writes x[:, :, scatter_col+1... wait: emits f32 h of column
            (scatter_col - q0) and scatters to x[:, :, scatter_col + 1]."""
            hcol = (wk.tile([P, 4, 1], F32, name=f"{pref}hc", tag="hcol")
                    if scatter_col is not None else None)

            def cb(ci, t1):
                nc.scalar.activation(ht[:, ci, q0:q0 + nq], t1[:], AF.Identity,
                                     scale=lnfg[:, ci:ci + 1],
                                     bias=lnfb[:, ci:ci + 1])
                if scatter_col is not None:
                    rel = scatter_col - q0
                    nc.scalar.activation(hcol[:, ci, :], t1[:, rel:rel + 1],
                                         AF.Identity,
                                         scale=lnfg[:, ci:ci + 1],
                                         bias=lnfb[:, ci:ci + 1])
            layernorm(lambda ci: x[:, ci, q0:q0 + nq], nq, cb, pref)
            if scatter_col is not None:
                tnew = scatter_col + 1
                for ci in range(4):
                    nc.vector.tensor_tensor(x[:, ci, tnew:tnew + 1],
                                            hcol[:, ci, :],
                                            wpelat[:, ci, tnew - L0:tnew - L0 + 1],
                                            ALU.add)

        # ================= decode step =================
        def decode_step(t, pref):
            M2 = t - 255          # keys in block 2 incl. self
            prow = t - 256        # vcache row in block 2
            # private residual column: x[:, :, t] must stay an EMBEDDING for
            # chunk B's recompute; decode's residual stream lives in xcol.
            xcol = wk.tile([P, 4, 1], F32, name=f"{pref}xc", tag="xcol")
            nc.vector.tensor_copy(xcol[:], x[:, :, t:t + 1])
            for li in range(NL):
                h16 = ln_to16(lambda ci: xcol[:, ci, :], 1,
                              W[f"ln1g{li}"], W[f"ln1b{li}"], f"{pref}l{li}n1")
                qkv = W[f"qkv{li}"]
                qd = wk.tile([P, 4, 1], F16, name=f"{pref}l{li}q", tag="qt16")
                for mb in range(8):
                    pmm = ps.tile([P, 1], F32, name=f"{pref}l{li}qk{mb}",
                                  tag="mm")
                    for ci in range(4):
                        nc.tensor.matmul(pmm[:],
                                         qkv[:, ci, mb * P:(mb + 1) * P],
                                         h16[:, ci, :], start=(ci == 0),
                                         stop=(ci == 3))
                    dst = (qd[:, mb, :] if mb < 4
                           else kc[li][:, mb - 4, t:t + 1])
                    nc.scalar.activation(dst, pmm[:], AF.Identity,
                                         bias=W[f"bqkv{li}"][:, mb:mb + 1])
                pv = ps.tile([1, 512], F32, name=f"{pref}l{li}v", tag="mm")
                for ci in range(4):
                    nc.tensor.matmul(pv[:], h16[:, ci, :],
                                     qkv[:, ci, 1024:1536],
                                     start=(ci == 0), stop=(ci == 3))
                vsl = vstage[li][:, :].rearrange("p (h e) -> p h e", h=8)
                nc.vector.tensor_tensor(vsl[:, :, 0:64], pv[:],
                                        W[f"bv{li}"][:], ALU.add)
                nc.sync.dma_start(vc[li][prow:prow + 1, 2, :], vstage[li][:, :])
                otd = wk.tile([P, 4, 1], F16, name=f"{pref}l{li}ot", tag="ot16")
                for h in range(NH):
                    cih, p0 = h // 2, (h % 2) * 64
                    etd = wk.tile([P, 3], F16, name=f"{pref}l{li}e{h}",
                                  tag="etd")
                    pso = ps.tile([65, 1], F32, name=f"{pref}l{li}o{h}",
                                  tag="attn_o")
                    for kb in range(2):
                        pss = ps.tile([P, 1], F32, name=f"{pref}l{li}s{h}{kb}",
                                      tag="attn_s")
                        nc.tensor.matmul(
                            pss[:], kc[li][p0:p0 + 64, cih, kb * P:(kb + 1) * P],
                            qd[p0:p0 + 64, cih, :], start=True, stop=True)
                        nc.scalar.activation(etd[:, kb:kb + 1], pss[:], AF.Exp,
                                             scale=0.125)
                        nc.tensor.matmul(pso[:],
                                         vc[li][:, kb, 65 * h:65 * h + 65],
                                         etd[:, kb:kb + 1], start=(kb == 0),
                                         stop=False)
                    pss2 = ps.tile([M2, 1], F32, name=f"{pref}l{li}s2{h}",
                                   tag="attn_s")
                    nc.tensor.matmul(pss2[:],
                                     kc[li][p0:p0 + 64, cih, 256:256 + M2],
                                     qd[p0:p0 + 64, cih, :], start=True,
                                     stop=True)
                    nc.scalar.activation(etd[0:M2, 2:3], pss2[:], AF.Exp,
                                         scale=0.125)
                    nc.tensor.matmul(pso[:],
                                     vc[li][0:M2, 2, 65 * h:65 * h + 65],
                                     etd[0:M2, 2:3], start=False, stop=True)
                    rec = wk.tile([1, 1], F32, name=f"{pref}l{li}rc{h}",
                                  tag="rec")
                    nc.vector.reciprocal(rec[:], pso[64:65, :])
                    ps_rb2 = ps.tile([64, 1], F32, name=f"{pref}l{li}rb{h}",
                                     tag="small")
                    nc.tensor.matmul(ps_rb2[:], onesr[0:1, 0:64], rec[:],
                                     start=True, stop=True)
                    rbc = wk.tile([64, 1], F32, name=f"{pref}l{li}rbc{h}",
                                  tag="rbc")
                    nc.vector.tensor_copy(rbc[:], ps_rb2[:])
                    nc.vector.tensor_tensor(otd[p0:p0 + 64, cih, :],
                                            pso[0:64, :], rbc[:], ALU.mult)
                for ci in range(4):
                    pmm = ps.tile([P, 1], F32, name=f"{pref}l{li}op{ci}",
                                  tag="mm")
                    for cik in range(4):
                        nc.tensor.matmul(
                            pmm[:], W[f"wo{li}"][:, cik, ci * P:(ci + 1) * P],
                            otd[:, cik, :], start=(cik == 0), stop=(cik == 3))
                    res = wk.tile([P, 1], F32, name=f"{pref}l{li}or{ci}",
                                  tag="resid")
                    nc.scalar.activation(res[:], pmm[:], AF.Identity,
                                         bias=W[f"bo{li}"][:, ci:ci + 1])
                    nc.vector.tensor_tensor(xcol[:, ci, :],
                                            xcol[:, ci, :], res[:], ALU.add)
                h2 = ln_to16(lambda ci: xcol[:, ci, :], 1,
                             W[f"ln2g{li}"], W[f"ln2b{li}"], f"{pref}l{li}n2")
                fc16 = wk.tile([P, 16, 1], F16, name=f"{pref}l{li}fc",
                               tag="fc16", bufs=1)
                zd = wk.tile([P, 16, 1], F32, name=f"{pref}l{li}zd", tag="zd")
                for mb in range(16):
                    pmm = ps.tile([P, 1], F32, name=f"{pref}l{li}fc{mb}",
                                  tag="mm")
                    for ci in range(4):
                        nc.tensor.matmul(
                            pmm[:], W[f"wfc{li}"][:, ci, mb * P:(mb + 1) * P],
                            h2[:, ci, :], start=(ci == 0), stop=(ci == 3))
                    nc.scalar.activation(zd[:, mb, :], pmm[:], AF.Identity,
                                         bias=W[f"bfc{li}"][:, mb:mb + 1])
                compose_gelu(zd[:].rearrange("p a b -> p (a b)"),
                             fc16[:].rearrange("p a b -> p (a b)"), 16,
                             f"{pref}l{li}")
                for ci in range(4):
                    pmm = ps.tile([P, 1], F32, name=f"{pref}l{li}pr{ci}",
                                  tag="mm")
                    for mb in range(16):
                        nc.tensor.matmul(
                            pmm[:], W[f"wpr{li}"][:, mb, ci * P:(ci + 1) * P],
                            fc16[:, mb, :], start=(mb == 0), stop=(mb == 15))
                    res = wk.tile([P, 1], F32, name=f"{pref}l{li}prr{ci}",
                                  tag="resid")
                    nc.scalar.activation(res[:], pmm[:], AF.Identity,
                                         bias=W[f"bpr{li}"][:, ci:ci + 1])
                    nc.vector.tensor_tensor(xcol[:, ci, :],
                                            xcol[:, ci, :], res[:], ALU.add)
            # lnf + scatter to x[:, :, t+1]
            hcol = wk.tile([P, 4, 1], F32, name=f"{pref}hc", tag="hcol")

            def cb(ci, t1):
                nc.scalar.activation(hcol[:, ci, :], t1[:], AF.Identity,
                                     scale=lnfg[:, ci:ci + 1],
                                     bias=lnfb[:, ci:ci + 1])
            layernorm(lambda ci: xcol[:, ci, :], 1, cb, f"{pref}f")
            for ci in range(4):
                nc.vector.tensor_tensor(x[:, ci, t + 1:t + 2], hcol[:, ci, :],
                                        wpelat[:, ci, t + 1 - L0:t + 2 - L0],
                                        ALU.add)

        # ================= vocab =================
        def vocab(tbs, pref):
            for vb in range(NVB):
                wt = vp.tile([P, 4, NV], F16, name=f"{pref}w{vb}", tag="wt")
                nc.sync.dma_start(wt[:], d_wte[:, vb, :, :])
                for tb in tbs:
                    pl = ps.tile([P, NV], F32, name=f"{pref}p{vb}_{tb}",
                                 tag="mm")
                    for ci in range(4):
                        nc.tensor.matmul(pl[:],
                                         ht[:, ci, tb * P:(tb + 1) * P],
                                         wt[:, ci, :], start=(ci == 0),
                                         stop=(ci == 3))
                    lt = vp.tile([P, NV], F32, name=f"{pref}l{vb}_{tb}",
                                 tag="lt", bufs=2)
                    nc.vector.tensor_copy(lt[:], pl[:])
                    nc.sync.dma_start(
                        d_logits[:, tb, vb * NV:(vb + 1) * NV], lt[:])
                    ex = vp.tile([P, NV], F16, name=f"{pref}e{vb}_{tb}",
                                 tag="ex", bufs=2)
                    nc.scalar.activation(ex[:], pl[:], AF.Exp,
                                         accum_out=seacc[:, tb, vb:vb + 1])

        # ================= emission =================
        for li in range(NL):
            chunk_layer(li, 0, 256, f"A{li}")
        chunk_lnf(0, 256, "Af", scatter_col=255)
        for t in range(256, 263):
            decode_step(t, f"D{t}")
        vocab([0, 1], "vA")
        for li in range(NL):
            chunk_layer(li, 256, 256, f"B{li}")
        chunk_lnf(256, 256, "Bf")
        vocab([2, 3], "vB")
        se = wk.tile([P, 4], F32, name="se_out", tag="se_out", bufs=1)
        import concourse.mybir as _mb
        nc.vector.tensor_reduce(se[:], seacc[:], axis=_mb.AxisListType.X,
                                op=ALU.add)
        nc.sync.dma_start(d_sumexp[:], se[:])
        if debug_outputs:
            nc.sync.dma_start(d_ht[:], ht[:])
            nc.sync.dma_start(d_x[:], x[:])

    nc.compile()
    return nc, names


# ---------------------------------------------------------------- runner

class _Runner:
    def __init__(self, nc, n_cores=NCORES):
        import jax
        import jax.numpy as jnp
        from jax.sharding import Mesh, PartitionSpec, NamedSharding
        from jax.experimental.shard_map import shard_map
        import concourse.mybir as mybir
        from concourse import bass2jax
        self.jax, self.jnp = jax, jnp
        bass2jax.install_neuronx_cc_hook()
        self.n_cores = n_cores
        partition_name = (nc.partition_id_tensor.name
                          if nc.partition_id_tensor else None)
        in_names, out_names, out_avals, zero_shapes = [], [], [], []
        for alloc in nc.m.functions[0].allocations:
            if not isinstance(alloc, mybir.MemoryLocationSet):
                continue
            name = alloc.memorylocations[0].name
            if alloc.kind == "ExternalInput":
                if name != partition_name:
                    in_names.append(name)
            elif alloc.kind == "ExternalOutput":
                shape = tuple(alloc.tensor_shape)
                dtype = mybir.dt.np(alloc.dtype)
                out_names.append(name)
                out_avals.append(jax.core.ShapedArray(shape, dtype))
                zero_shapes.append((shape, dtype))
        self.in_names, self.out_names = in_names, out_names
        self.zero_shapes = zero_shapes
        n_params, n_outs = len(in_names), len(out_names)
        all_in = list(in_names) + list(out_names)
        if partition_name is not None:
            all_in.append(partition_name)

        def _body(*args):
            operands = list(args)
            if partition_name is not None:
                operands.append(bass2jax.partition_id_tensor())
            outs = bass2jax._bass_exec_p.bind(
                *operands, out_avals=tuple(out_avals), in_names=tuple(all_in),
                out_names=tuple(out_names), lowering_input_output_aliases=(),
                sim_require_finite=False, sim_require_nnan=False, nc=nc)
            return tuple(outs)

        devices = jax.devices()[:n_cores]
        self.mesh = Mesh(np.asarray(devices), ("core",))
        in_specs = (PartitionSpec("core"),) * (n_params + n_outs)
        out_specs = (PartitionSpec("core"),) * n_outs
        donate = tuple(range(n_params, n_params + n_outs))
        self.fn = jax.jit(
            shard_map(_body, mesh=self.mesh, in_specs=in_specs,
                      out_specs=out_specs, check_rep=False),
            donate_argnums=donate, keep_unused=True)
        self.sharding = NamedSharding(self.mesh, PartitionSpec("core"))

    def put_inputs(self, in_maps):
        concat = [np.concatenate([np.asarray(in_maps[c][n])
                                  for c in range(self.n_cores)], axis=0)
                  for n in self.in_names]
        return [self.jax.device_put(a, self.sharding) for a in concat]

    def zeros(self):
        return [self.jax.device_put(
            self.jnp.zeros((self.n_cores * s[0], *s[1:]), d), self.sharding)
            for (s, d) in self.zero_shapes]

    def run(self, dev_inputs):
        outs = self.fn(*dev_inputs, *self.zeros())
        self.jax.block_until_ready(outs)
        return outs

    def outs_to_maps(self, outs):
        res = [dict() for _ in range(self.n_cores)]
        for i, name in enumerate(self.out_names):
            a = np.asarray(outs[i])
            a = a.reshape(self.n_cores, a.shape[0] // self.n_cores,
                          *a.shape[1:])
            for c in range(self.n_cores):
                res[c][name] = a[c]
        return res


# ---------------------------------------------------------------- host post

def assemble(outs_maps, names, labels):
    """outs_maps: per-core dict tensorname->np. Returns (loss, logits)."""
    logits = np.empty((B, S, V), np.float32)
    sumexp = np.empty((B, S), np.float64)
    for r in range(B):
        halves, se = [], []
        for half in range(2):
            m = outs_maps[2 * r + half]
            lg = m[names["logits"]]                   # [128, 4, 16000]
            halves.append(lg.transpose(1, 0, 2).reshape(S, VH))
            se.append(m[names["sumexp"]].transpose(1, 0).reshape(S))
        logits[r] = np.concatenate(halves, axis=1)
        sumexp[r] = se[0].astype(np.float64) + se[1].astype(np.float64)
    labels = np.asarray(labels)
    shift_labels = labels[:, 1:]
    valid = shift_labels != -100
    lse = np.log(sumexp[:, :-1])                      # [B, 511]
    tgt = np.where(valid, shift_labels, 0)
    tl = np.take_along_axis(logits[:, :-1].astype(np.float64),
                            tgt[..., None], axis=-1)[..., 0]
    nll = lse - tl
    loss = (nll * valid).sum() / valid.sum()
    return np.float32(loss), logits


def kernel(input_ids, attention_mask, labels, position_ids, params):
    if "runner" not in _CACHE:
        nc, names = build_nc(debug_outputs=False)
        _CACHE["nc"], _CACHE["names"] = nc, names
        _CACHE["runner"] = _Runner(nc)
    r, names = _CACHE["runner"], _CACHE["names"]
    in_maps = pack_inputs(input_ids, attention_mask, labels, position_ids,
                          params)
    in_maps = [{names[k]: v for k, v in m.items()} for m in in_maps]
    dev_in = r.put_inputs(in_maps)
    outs = r.run(dev_in)
    return assemble(r.outs_to_maps(outs), names, labels)
